# revision 1
# baseline (speedup 1.0000x reference)
"""MoE feed-forward (8 experts, top-2) on 8 Trainium2 NeuronCores.

Expert-parallel: core e holds expert e's weights. The (cheap) router runs on
host; tokens are dispatched to expert cores host-side with capacity factor
1.0 (exactly T*K/E = 2048 assignments per core; the ~0.7% overflow is
computed exactly on host). Each core runs
  yT = gelu(x @ W1 + b1) @ W2
for its tokens; the host applies the combine weight cw and b2 during the
scatter-add back into token order.

Device schedule: both GEMMs run tokens-as-moving-dim in 4 x 512-token
blocks at the fp16 PE streaming floor (512 cycles/token) with the minimum
PE instruction count (one Ldweights+Matmult pair per 512-token tile);
block 0's GEMM1 runs k-outer within m-groups of 8 (8 psum banks) so its
matmuls consume k-major weight/activation slices at DMA arrival rate;
dummy warm-up matmuls cover the remaining initial DMA window so the PE
clock (HAM) is at full rate when real work starts; GEMM2 PSUM is evacuated by a single DVE copy per tile straight
to the fp16 output DMA.

Device numerics: matmul operands fp16 (PSUM accumulation fp32, GELU fp32,
output DMA fp16) -> ~5e-4 relative error overall.

Fixed problem shape (hardcoded per contest contract):
  x [4, 2048, 1024], Wr [8, 1024], W1 [8, 1024, 4096], b1 [8, 4096],
  W2 [8, 4096, 1024], b2 [8, 1024]. TOP_K = 2.
"""

import math
import os

import numpy as np

import concourse.bass as bass
import concourse.mybir as mybir
import concourse.tile as tile
from concourse import bacc
from concourse.bass import ts
from concourse.bass_utils import run_bass_kernel_spmd

D = 1024  # d_model
F = 4096  # ff dim
E = 8  # experts == cores
TOP_K = 2
# Capacity factor 1.0: each expert-core processes exactly T*K/E = 2048
# assignments; the (~0.7% for seed 0) overflow is computed exactly on host.
CAP = 2048
# token blocks (moving free dim): 4 x 512 minimizes the PE instruction
# count (each Ldweights+Matmult pair costs ~2.5-5ns of NX issue on HW)
BLOCKS = [(0, 512), (512, 512), (1024, 512), (1536, 512)]
BLK = 512
WARM_MMS = 10  # ends ~3.1us: matches the central HW estimate for first-slice arrival (parallel HWDGE dispatch + faster ack than the cost model); undersizing is cheap, oversizing delays real work 1:1
KD = D // 128  # 8   contraction tiles for GEMM1
KF = F // 128  # 32  contraction tiles for GEMM2
MF = F // 128  # 32  output tiles for GEMM1
MD = D // 128  # 8   output tiles for GEMM2

F32 = mybir.dt.float32
F16 = mybir.dt.float16

# Cache the built+finalized Bass graph across kernel() calls in one process.
_NC_CACHE = {}

LAST_RESULT = None  # BassKernelResults of the most recent device run


def _build_nc():
    nc = bacc.Bacc("TRN2", target_bir_lowering=False)

    xT = nc.declare_dram_parameter("xT", [128, KD, CAP], F16, isOutput=False)
    # w1 pre-chunked k-major on host: [k, p, m, q] so block-0 can consume
    # contraction slices in arrival order
    w1 = nc.declare_dram_parameter("w1", [KD, 128, MF, 128], F16, isOutput=False)
    b1c = nc.declare_dram_parameter("b1c", [128, MF], F32, isOutput=False)
    w2 = nc.declare_dram_parameter("w2", [MD // 4, 128, 4, KF, 128], F16, isOutput=False)
    # device emits raw h @ W2 (b2 and the combine weight cw are applied on
    # host during the scatter-add — that keeps the device output path to a
    # single DVE copy per psum tile)
    yT = nc.declare_dram_parameter("yT", [128, MD, CAP], F16, isOutput=True)

    with tile.TileContext(nc) as tc:
        with (
            tc.tile_pool(name="w1p", bufs=1) as w1pool,
            tc.tile_pool(name="w2p", bufs=1) as w2pool,
            tc.tile_pool(name="hp", bufs=1) as hpool,
            tc.tile_pool(name="xbp", bufs=4) as xbpool,
            tc.tile_pool(name="stage", bufs=3) as spool,
            tc.tile_pool(name="const", bufs=1) as cpool,
            tc.tile_pool(name="ps", bufs=8, space="PSUM") as pspool,
        ):
            # PE p-state/HAM warm-up: dummy matmuls on a scratch tile fill the
            # otherwise-idle window while the first weight/activation DMAs
            # land, so real matmuls start at full clock. Results are never
            # read; the scratch input is zeroed to keep the data path clean.
            warm_x = cpool.tile([128, BLK], F16)
            # only the lhsT columns need defined data; the streamed columns
            # feed matmuls whose results are never read
            nc.gpsimd.memset(warm_x[:, :128], 0.0)
            warm_ps = pspool.tile([128, BLK], F32, tag="ps")
            for _ in range(WARM_MMS):
                nc.tensor.matmul(
                    warm_ps[:, :256], lhsT=warm_x[:, :128], rhs=warm_x[:, :256],
                    start=True, stop=True,
                )

            # Startup: block-0's GEMM1 runs k-outer within m-groups of 8, so
            # it only needs (w1[k, m0-7], xb0[k]) pairs in k order. Interleave
            # those small deliveries on the two HWDGE queues; the matmuls
            # consume them at arrival rate. The rest of w1 follows k-major.
            bw0 = BLOCKS[0][1]
            w1t = w1pool.tile([128, KD, MF, 128], F16, tag="w1")
            xb0 = xbpool.tile([128, KD, BLK], F16, tag="xb")
            # k0's weights split 2+6 so the first matmuls (m0-1) gate on a
            # 64KB piece instead of 256KB
            nc.sync.dma_start(
                out=w1t[:, 0, 0:2].rearrange("p m q -> p (m q)"),
                in_=w1[0, :, 0:2].rearrange("p m q -> p (m q)"),
            )
            nc.scalar.dma_start(out=xb0[:, 0, :bw0], in_=xT[:, 0, 0:bw0])
            nc.sync.dma_start(
                out=w1t[:, 0, 2:8].rearrange("p m q -> p (m q)"),
                in_=w1[0, :, 2:8].rearrange("p m q -> p (m q)"),
            )
            for k in range(1, KD):
                nc.sync.dma_start(
                    out=w1t[:, k, 0:8].rearrange("p m q -> p (m q)"),
                    in_=w1[k, :, 0:8].rearrange("p m q -> p (m q)"),
                )
                nc.scalar.dma_start(out=xb0[:, k, :bw0], in_=xT[:, k, 0:bw0])
            # rest of w1 ordered by consuming m-group: grp1's slices across
            # all k land before grp2's, matching the k-outer group schedule
            for mg in range(1, 4):
                for k in range(KD):
                    nc.sync.dma_start(
                        out=w1t[:, k, 8 * mg : 8 * mg + 8].rearrange(
                            "p m q -> p (m q)"
                        ),
                        in_=w1[k, :, 8 * mg : 8 * mg + 8].rearrange(
                            "p m q -> p (m q)"
                        ),
                    )
            b1t = cpool.tile([128, MF], F32)
            nc.scalar.dma_start(out=b1t[:], in_=b1c[:])
            # w2 slices are issued below, spread across early blocks
            w2t = w2pool.tile([128, MD, KF, 128], F16, tag="w2")

            for bi, (off, bw) in enumerate(BLOCKS):
                if off == 0:
                    xb = xb0
                else:
                    xb = xbpool.tile([128, KD, BLK], F16, tag="xb")
                    nc.sync.dma_start(
                        out=xb[:, :, :bw], in_=xT[:, :, off : off + bw]
                    )

                # ---- GEMM1: h = gelu(W1.T @ x + b1), h stays in SBUF ----
                h = hpool.tile([128, MF, BLK], F16, tag="h")
                if bi == 0:
                    # k-outer within m-groups of 8 (8 psum banks): each k-step
                    # needs only the k-th w1/x slices, matching DMA arrival
                    for grp in range(MF // 8):
                        for r in range(2):
                            # w2 slices interleave with compute as before
                            mw = 2 * grp + r
                            g2, r2 = divmod(mw, 4)
                            nc.sync.dma_start(
                                out=w2t[:, mw].rearrange("p k q -> p (k q)"),
                                in_=w2[g2, :, r2].rearrange("p k q -> p (k q)"),
                            )
                        pss = []
                        for mi in range(8):
                            ps_g = pspool.tile([128, BLK], F32, tag="ps")
                            pss.append(ps_g)
                        for k in range(KD):
                            for mi in range(8):
                                nc.tensor.matmul(
                                    pss[mi][:, :bw],
                                    lhsT=w1t[:, k, 8 * grp + mi],
                                    rhs=xb[:, k, :bw],
                                    start=(k == 0),
                                    stop=(k == KD - 1),
                                )
                        for mi in range(8):
                            m = 8 * grp + mi
                            nc.scalar.activation(
                                h[:, m, :bw],
                                pss[mi][:, :bw],
                                mybir.ActivationFunctionType.Gelu,
                                bias=b1t[:, m : m + 1],
                            )
                else:
                    for m in range(MF):
                        ps = pspool.tile([128, BLK], F32, tag="ps")
                        for k in range(KD):
                            nc.tensor.matmul(
                                ps[:, :bw],
                                lhsT=w1t[:, k, m],
                                rhs=xb[:, k, :bw],
                                start=(k == 0),
                                stop=(k == KD - 1),
                            )
                        nc.scalar.activation(
                            h[:, m, :bw],
                            ps[:, :bw],
                            mybir.ActivationFunctionType.Gelu,
                            bias=b1t[:, m : m + 1],
                        )

                # ---- GEMM2: yT = W2.T @ h  (cw scale + b2 applied on host) ----
                for m in range(MD):
                    # the very last m-tile runs as two 256-column halves so the
                    # copy+DMA of the first half overlaps the second half's
                    # matmuls, shortening the post-matmul tail (256-wide MMs
                    # still stream slower than LDWEIGHTS, so no LDW binding)
                    if bi == len(BLOCKS) - 1 and m == MD - 1:
                        chunks = [(0, 256), (256, 256)]
                    else:
                        chunks = [(0, bw)]
                    for c0, cl in chunks:
                        ps = pspool.tile([128, BLK], F32, tag="ps")
                        for k in range(KF):
                            nc.tensor.matmul(
                                ps[:, :cl],
                                lhsT=w2t[:, m, k],
                                rhs=h[:, k, c0 : c0 + cl],
                                start=(k == 0),
                                stop=(k == KF - 1),
                            )
                        ys = spool.tile([128, BLK], F16, tag="ys")
                        nc.vector.tensor_copy(out=ys[:, :cl], in_=ps[:, :cl])
                        nc.sync.dma_start(
                            out=yT[:, m, off + c0 : off + c0 + cl], in_=ys[:, :cl]
                        )

    nc.finalize()
    return nc


def _gelu_exact_np(x):
    try:
        from scipy.special import erf

        return 0.5 * x * (1.0 + erf(x / np.sqrt(2.0)))
    except ImportError:
        _erf = np.vectorize(math.erf)
        return 0.5 * x * (1.0 + _erf(x / np.sqrt(2.0)))


def _route(t, Wr):
    """Replicate the reference router in fp32 numpy: softmax + top-2 with
    jax.lax.top_k tie-breaking (first index wins), weights renormalized."""
    logits = t @ Wr.T  # [T, E] fp32
    mx = logits.max(axis=1, keepdims=True)
    ez = np.exp(logits - mx, dtype=np.float32)
    probs = ez / ez.sum(axis=1, keepdims=True, dtype=np.float32)

    arange = np.arange(t.shape[0])
    i1 = probs.argmax(axis=1)
    masked = probs.copy()
    masked[arange, i1] = -np.inf
    i2 = masked.argmax(axis=1)
    v1 = probs[arange, i1]
    v2 = probs[arange, i2]
    s = v1 + v2
    return i1, i2, v1 / s, v2 / s


def kernel(x, Wr, W1, b1, W2, b2):
    global LAST_RESULT

    x = np.asarray(x, dtype=np.float32)
    Wr = np.asarray(Wr, dtype=np.float32)
    W1 = np.asarray(W1, dtype=np.float32)
    b1 = np.asarray(b1, dtype=np.float32)
    W2 = np.asarray(W2, dtype=np.float32)
    b2 = np.asarray(b2, dtype=np.float32)

    Bb, Ss, _ = x.shape
    T = Bb * Ss
    t = np.ascontiguousarray(x.reshape(T, D))

    i1, i2, cw1, cw2 = _route(t, Wr)

    # per-expert token lists (device handles first CAP; remainder -> host)
    dev_idx, dev_cw, host_idx, host_cw = [], [], [], []
    for e in range(E):
        sel1 = np.nonzero(i1 == e)[0]
        sel2 = np.nonzero(i2 == e)[0]
        idx = np.concatenate([sel1, sel2])
        cw = np.concatenate([cw1[sel1], cw2[sel2]]).astype(np.float32)
        dev_idx.append(idx[:CAP])
        dev_cw.append(cw[:CAP])
        host_idx.append(idx[CAP:])
        host_cw.append(cw[CAP:])

    in_maps = []
    for e in range(E):
        idx = dev_idx[e]
        n = len(idx)
        xe = np.zeros((128, KD, CAP), dtype=np.float16)
        # t[idx] : [n, D] -> [n, KD, 128] -> [128, KD, n]
        xe[:, :, :n] = t[idx].reshape(n, KD, 128).transpose(2, 1, 0)
        # W1[e]: [D, F] -> [m][p][k][q] with row index k*128+p, col index m*128+q
        # [K,128p,G,4m,128q] -> [G, p, m, K, q]
        w1e = np.ascontiguousarray(
            W1[e].reshape(KD, 128, MF, 128), dtype=np.float16
        )
        w2e = np.ascontiguousarray(
            W2[e].reshape(KF, 128, MD // 4, 4, 128).transpose(2, 1, 3, 0, 4),
            dtype=np.float16,
        )
        in_maps.append(
            {
                "xT": xe,
                "w1": w1e,
                "b1c": np.ascontiguousarray(b1[e].reshape(MF, 128).T),
                "w2": w2e,
            }
        )

    if "nc" not in _NC_CACHE:
        _NC_CACHE["nc"] = _build_nc()
    nc = _NC_CACHE["nc"]

    try:
        res = run_bass_kernel_spmd(nc, in_maps, core_ids=list(range(E)))
    except ModuleNotFoundError:
        # BASS_TRACE was requested but this environment lacks the axon NTFF
        # profiling hook module; rerun with tracing disabled
        os.environ["BASS_NEVER_TRACE"] = "1"
        res = run_bass_kernel_spmd(nc, in_maps, core_ids=list(range(E)))
    LAST_RESULT = res

    out = np.zeros((T, D), dtype=np.float32)
    for e in range(E):
        idx = dev_idx[e]
        n = len(idx)
        if n == 0:
            continue
        yT = res.results[e]["yT"].astype(np.float32)  # [128, MD, CAP] = raw h @ W2
        ye = yT.transpose(2, 1, 0).reshape(CAP, D)[:n]  # [n, D]
        out[idx] += dev_cw[e][:, None] * (ye + b2[e][None, :])

    # exact host fallback for (rare) capacity overflow
    for e in range(E):
        idx = host_idx[e]
        if len(idx) == 0:
            continue
        h = _gelu_exact_np(t[idx] @ W1[e] + b1[e]).astype(np.float32)
        ye = (h @ W2[e] + b2[e]) * host_cw[e][:, None]
        out[idx] += ye.astype(np.float32)

    return out.reshape(Bb, Ss, D)

